# revision 27
# baseline (speedup 1.0000x reference)
"""Causal multi-head attention block (QKV proj -> causal attention -> out proj)
for Trainium2, distributed over 8 NeuronCores.

Sharding: core c handles batch b = c//2 and head-group g = c%2 (8 of 16 heads).
Each core computes qkv for its group's columns of w_attn, runs causal attention
for its 8 heads, and multiplies by its group's rows of w_proj, producing a
partial y[b]. The host sums the two partials per batch and adds b_proj.

All device matmuls run in bf16 (PSUM accumulation stays fp32; ~1e-3 rel err,
comfortably under the 2e-2 gate). bf16 halves HBM traffic vs fp32 and removes
the fp32r 4x penalty on <256-wide moving operands. The kernel works in
transposed layouts end-to-end (host passes x[b].T as bf16, device returns
y[b].T as bf16) so no on-device transposes are needed:
  q^T,k^T = w_{q,k}^T-chunks @ x^T      [cols, tok]
  s^T     = k_h^T-chunks    @ q_h^T     [k_tok, q_tok]  (exp on ACT -> p^T,
            causal tri-mask multiply on the idle GPSIMD engine)
  out^T   = [v_h | 1x64]^T  @ p^T       [128, q_tok]: v is padded with 64 ones
            columns so the pv matmul lands the softmax denominators
            REPLICATED on psum partitions 64..127 -- normalization is then
            a fast-approx reciprocal + one multiply on DVE, with no
            partition-broadcast op at all
  y^T     = w_proj-chunks   @ out_norm^T

Schedule: the scores for pairs of causal k-blocks land adjacently in one
2-bank PSUM tile so a single ACT exp covers both (the per-instruction ACT
overhead is ~185ns, and ACT is the co-critical engine during attention).
Attention emission pipelines one block-group ahead (scores of group g+1 are
in the PE queue before the pv of group g, hiding the exp->mask chain), and a
deque of filler closures (next pair's q/k projection matmuls, then the
token-half-0 output projections) is drained into the remaining exp-bound PE
gaps. The v projection runs dc-outer over 8 concurrent PSUM accumulators so
the PE streams right behind the chunked input DMA.
"""

import math
import sys
from collections import deque

import numpy as np

if "/opt/trn_rl_repo" not in sys.path:
    sys.path.insert(0, "/opt/trn_rl_repo")

B, S, D = 4, 1024, 1024
H = 16
HPG = 8              # heads per group (2 groups of 8)
hd = D // H          # 64
GC = HPG * hd        # 512 cols per group for each of q,k,v
P = 128
DC = D // P          # 8 contraction chunks

_CACHE = {}


def _build(repeat=1, ones_cols=64, pt_bufs=6, yt_bufs=4, small_bufs=4,
           xw_chunks=4, mask_pool=True, tail_alt=True, drain_k=1,
           split_sched=False, qk_evict_dve=True, norm_pool=False,
           warmup=0, proj1_dbl=True):
    import concourse.mybir as mybir
    import concourse.tile as tile
    from concourse import bacc
    from concourse.masks import make_upper_triangular

    f32 = mybir.dt.float32
    bf16 = mybir.dt.bfloat16
    Exp = mybir.ActivationFunctionType.Exp
    mult = mybir.AluOpType.mult

    nc = bacc.Bacc("TRN2", target_bir_lowering=False, debug=False, num_devices=8)
    xT = nc.dram_tensor("xT", [D, S], bf16, kind="ExternalInput").ap()
    wq = nc.dram_tensor("wq", [D, GC], bf16, kind="ExternalInput").ap()
    wk = nc.dram_tensor("wk", [D, GC], bf16, kind="ExternalInput").ap()
    wv = nc.dram_tensor("wv", [D, GC], bf16, kind="ExternalInput").ap()
    wp = nc.dram_tensor("wp", [GC, D], bf16, kind="ExternalInput").ap()
    yT = nc.dram_tensor("yT", [D, S], bf16, kind="ExternalOutput").ap()

    scale = 1.0 / math.sqrt(hd)
    vw = hd + ones_cols          # v columns per head incl. ones block

    with tile.TileContext(nc) as tc:
        with tc.tile_pool(name="const", bufs=1) as const, \
             tc.tile_pool(name="big", bufs=1) as big, \
             tc.tile_pool(name="pt", bufs=pt_bufs) as ptp, \
             tc.tile_pool(name="small", bufs=small_bufs) as small, \
             tc.tile_pool(name="yt", bufs=yt_bufs) as ytp, \
             tc.tile_pool(name="ps", bufs=2, space="PSUM") as ps:
          # PSUM budget (8 banks): tag "sp" 2x[P,1024] (4 banks) score tiles,
          # tag "qk" 1x[P,1024] (2 banks) q/k projection acc, tag "acc"
          # 2x[P,512] (2 banks) attention/out-projection accumulators.

          for _rep in range(repeat):
            tri = const.tile([P, P], bf16, tag="tri")    # keep iff k_local <= q_local
            make_upper_triangular(nc, tri[:], val=1.0, diag=True)
            if warmup:
                # input-independent matmuls issued at t~0 start the PE
                # activity monitor's clock-ungate window early, so the real
                # matmuls hit full clock sooner
                warm = ps.tile([P, 512], f32, tag="acc", name="warm")
                for i in range(warmup):
                    nc.tensor.matmul(warm[:, 0:P], tri[:], tri[:],
                                     start=(i == 0), stop=(i == warmup - 1))

            # chunked input loads so compute can start on early chunks
            xt = big.tile([P, DC, S], bf16, tag="xt")
            xTr = xT.rearrange("(dc p) t -> p dc t", p=P)
            wqt = big.tile([P, DC, GC], bf16, tag="wq")
            wqr = wq.rearrange("(dc p) c -> p dc c", p=P)
            wkt = big.tile([P, DC, GC], bf16, tag="wk")
            wkr = wk.rearrange("(dc p) c -> p dc c", p=P)
            wvt = big.tile([P, DC, GC], bf16, tag="wv")
            wvr = wv.rearrange("(dc p) c -> p dc c", p=P)
            # first chunks are single-dc (the leading x chunk split again by
            # token half) so the v projection starts as early as possible;
            # later chunks batch up to amortize DMA issue overhead
            nc.sync.dma_start(xt[:, 0:1, 0:512], xTr[:, 0:1, 0:512])
            nc.sync.dma_start(wvt[:, 0:1, :], wvr[:, 0:1, :])
            nc.sync.dma_start(xt[:, 0:1, 512:1024], xTr[:, 0:1, 512:1024])
            for c0, st in [(1, 1), (2, 2), (4, 2), (6, 2)]:
                nc.sync.dma_start(xt[:, c0:c0 + st, :], xTr[:, c0:c0 + st, :])
                nc.sync.dma_start(wvt[:, c0:c0 + st, :], wvr[:, c0:c0 + st, :])
            nc.sync.dma_start(wqt[:], wqr[:])
            nc.sync.dma_start(wkt[:], wkr[:])
            wpt_c = big.tile([P, GC // P, D], bf16, tag="wp")
            wpr = wp.rearrange("(cc p) o -> p cc o", p=P)
            nc.sync.dma_start(wpt_c[:], wpr[:])

            # q^T/k^T for the group: [col(128), chunk, tok]; chunks 0-3 = q, 4-7 = k
            qkt = big.tile([P, 2 * GC // P, S], bf16, tag="qkt")
            # v padded with 64 ones columns per head: the pv matmul then
            # lands sum(p) replicated on psum partitions 64..127, so softmax
            # normalization needs no partition broadcast at all.
            vaug = big.tile([P, S // P, HPG, vw], bf16, tag="vaug")
            nc.gpsimd.memset(vaug[:, :, :, hd:vw], 1.0)
            # normalized attention output ^T: [chan(128), chan_chunk, tok]
            outt = big.tile([P, GC // P, S], bf16, tag="outt")

            # ---- v projection: dc-outer over 8 concurrent PSUM accs ----
            vacc_d = [ps.tile([P, 1024], f32, tag="sp", name=f"vd{i}")
                      for i in range(2)]
            vacc_q = [ps.tile([P, 512], f32, tag="qk", name=f"vq{i}", bufs=2)
                      for i in range(2)]
            vacc_s = [ps.tile([P, 512], f32, tag="acc", name=f"vs{i}")
                      for i in range(2)]

            def vacc(t8):
                if t8 < 4:
                    return vacc_d[t8 // 2][:, (t8 % 2) * 512:(t8 % 2 + 1) * 512]
                if t8 < 6:
                    return vacc_q[t8 - 4][:]
                return vacc_s[t8 - 6][:]

            for dc in range(DC):
                for t8 in range(S // P):
                    nc.tensor.matmul(
                        vacc(t8),
                        xt[:, dc, t8 * P:(t8 + 1) * P],
                        wvt[:, dc, :],
                        start=(dc == 0), stop=(dc == DC - 1),
                    )
            # eviction priority: "qk"-tag bufs first on DVE (the pair-0 q/k
            # projection needs them immediately), sp/acc-tag bufs on ACT
            for i in range(2):
                nc.vector.tensor_copy(
                    out=vaug[:, 4 + i, :, 0:hd],
                    in_=vacc_q[i][:].rearrange("p (h j) -> p h j", h=HPG))
            for i in range(2):
                nc.scalar.copy(
                    vaug[:, 2 * i:2 * i + 2, :, 0:hd],
                    vacc_d[i][:].rearrange("p (t h j) -> p t h j", t=2, h=HPG))
            for i in range(2):
                nc.scalar.copy(
                    vaug[:, 6 + i, :, 0:hd],
                    vacc_s[i][:].rearrange("p (h j) -> p h j", h=HPG))

            # ---- filler machinery: closures drained into attention PE gaps
            fillers = deque()

            def drain(k):
                for _ in range(k):
                    if not fillers:
                        return
                    fillers.popleft()()

            def flush():
                while fillers:
                    fillers.popleft()()

            # ---- q/k projections (one [P,1024] psum acc per column group,
            # both token halves; single merged 1024-wide ACT eviction) ----
            def qk_closures(hp):
                fs = []
                for cc8 in (hp, 4 + hp):
                    src = wqt if cc8 < 4 else wkt
                    cbase = (cc8 % 4) * P
                    for t5 in (0, 1):
                        cell = []
                        for dc0 in range(0, DC, 2):
                            def f(src=src, cbase=cbase, t5=t5, dc0=dc0,
                                  cc8=cc8, cell=cell):
                                if not cell:
                                    cell.append(ps.tile([P, 512], f32,
                                                        tag="qk", bufs=2,
                                                        name=f"qk{cc8}{t5}"))
                                acc = cell[0]
                                for dc in (dc0, dc0 + 1):
                                    nc.tensor.matmul(
                                        acc[:],
                                        src[:, dc, cbase:cbase + P],
                                        xt[:, dc, t5 * 512:(t5 + 1) * 512],
                                        start=(dc == 0), stop=(dc == DC - 1),
                                    )
                            fs.append(f)

                        # evict each token-half as soon as its accumulation
                        # completes: qc=0 attention only needs the t5=0 half
                        def fe(cc8=cc8, cell=cell, t5=t5):
                            sl = slice(t5 * 512, (t5 + 1) * 512)
                            if qk_evict_dve:
                                nc.vector.tensor_copy(out=qkt[:, cc8, sl],
                                                      in_=cell[0][:])
                            else:
                                nc.scalar.copy(qkt[:, cc8, sl], cell[0][:])
                        fs.append(fe)
                return fs

            # ---- output projection; t5=0 groups are emitted as fillers into
            # the last pair's attention, t5=1 as the dense tail ----
            def proj_group(t5, oc):
                acc = ps.tile([P, 512], f32, tag="acc", name=f"pj{t5}{oc}")
                for cc in range(GC // P):
                    nc.tensor.matmul(
                        acc[:],
                        wpt_c[:, cc, oc * P:(oc + 1) * P],
                        outt[:, cc, t5 * 512:(t5 + 1) * 512],
                        start=(cc == 0), stop=(cc == GC // P - 1),
                    )
                yt = ytp.tile([P, 512], bf16, tag="yt", name=f"yt{t5}{oc}")
                if t5 == 0 or (tail_alt and oc % 2 == 1):
                    nc.vector.tensor_copy(out=yt[:], in_=acc[:])
                else:
                    nc.scalar.copy(yt[:], acc[:])
                nc.sync.dma_start(
                    yT[oc * P:(oc + 1) * P, t5 * 512:(t5 + 1) * 512], yt[:])

            def proj_closures(ocs):
                # one atomic closure per oc block: keeps the "acc"-tag psum
                # lifetime short so the attention accs' rotation never starves
                fs = []
                for oc in ocs:
                    def f(oc=oc):
                        proj_group(0, oc)
                    fs.append(f)
                return fs

            # ---- causal attention, transposed layouts. Block groups share
            # one 2-bank psum tile + one exp; scores pipeline a group ahead.
            def attn(h, qc):
                prow = 64 * (h % 2)
                qh = qkt[prow:prow + hd, h // 2, :]
                kh = qkt[prow:prow + hd, 4 + h // 2, :]
                acc = ps.tile([P, 512], f32, tag="acc", name=f"at{h}{qc}")
                # groups of (k-block, local col offset, width); widths follow
                # causality: width = 512 - 128*max(0, kb - 4qc); adjacent
                # blocks pack so the exp span has no gap
                if qc == 0:
                    groups = [[(0, 0, 512), (1, 512, 384)],
                              [(2, 0, 256), (3, 256, 128)]]
                else:
                    groups = [[(4, 0, 512), (0, 512, 512)],
                              [(1, 0, 512), (2, 512, 512)],
                              [(3, 0, 512), (5, 512, 384)],
                              [(6, 0, 256), (7, 256, 128)]]
                n = len(groups)
                sps, pts = [None] * n, [None] * n

                def scores(gi):
                    sp = ps.tile([P, 1024], f32, tag="sp", name=f"sp{h}{qc}{gi}")
                    for kb, loc, width in groups[gi]:
                        nc.tensor.matmul(
                            sp[:, loc:loc + width],
                            kh[:, kb * P:(kb + 1) * P],
                            qh[:, qc * 512 + 512 - width:(qc + 1) * 512],
                            start=True, stop=True,
                        )
                    sps[gi] = sp

                def softmax_part(gi):
                    grp = groups[gi]
                    span = grp[-1][1] + grp[-1][2]
                    pt = ptp.tile([P, 1024], bf16, tag="pt", name=f"pt{h}{qc}{gi}")
                    nc.scalar.activation(pt[:, :span], sps[gi][:, :span],
                                         Exp, scale=scale)
                    for kb, loc, width in grp:
                        if kb - 4 * qc >= 0:       # diagonal: triangular mask
                            eng = nc.gpsimd if mask_pool else nc.vector
                            eng.tensor_tensor(
                                pt[:, loc:loc + P], pt[:, loc:loc + P],
                                tri[:], mult)
                    pts[gi] = pt

                def pv(gi):
                    grp = groups[gi]
                    for j, (kb, loc, width) in enumerate(grp):
                        nc.tensor.matmul(
                            acc[:vw, 512 - width:512],
                            vaug[:, kb, h, :],
                            pts[gi][:, loc:loc + width],
                            start=(gi == 0 and j == 0),
                            stop=(gi == n - 1 and j == len(grp) - 1),
                        )

                scores(0)
                softmax_part(0)
                for gi in range(n):
                    drain(drain_k)
                    if gi + 1 < n:
                        scores(gi + 1)
                        softmax_part(gi + 1)
                    drain(1)
                    pv(gi)
                # normalize: psum rows 64..127 hold rowsum replicated 64x
                # (from vaug's ones block) -> reciprocal + multiply
                rsb = small.tile([ones_cols, 512], f32, tag="rsb",
                                 name=f"rsb{h}{qc}")
                # ~18-bit reciprocal, ~5x faster than the exact
                # InstReciprocal; denominators are benign sums.
                # (custom-DVE op misreads PSUM: stage via SBUF)
                rss = small.tile([ones_cols, 512], f32, tag="rss",
                                 name=f"rss{h}{qc}")
                nc.vector.tensor_copy(out=rss[:], in_=acc[hd:hd + ones_cols, :])
                nc.vector.reciprocal_approx_fast(out=rsb[:], in_=rss[:])
                norm_eng = nc.gpsimd if norm_pool else nc.vector
                for s0 in range(0, hd, ones_cols):
                    n0 = min(ones_cols, hd - s0)
                    norm_eng.tensor_tensor(
                        outt[prow + s0:prow + s0 + n0, h // 2,
                             qc * 512:(qc + 1) * 512],
                        acc[s0:s0 + n0, :], rsb[:n0, :], mult)

            # ---- emission schedule ----
            # phase A0: q/k projections (next pair's as fillers) + qc=0
            # attention; phase A1: qc=1 attention with the token-half-0
            # output projections spread across it as fillers; dense
            # token-half-1 projection tail.
            for f in qk_closures(0):
                f()
            if split_sched:
                for hp in range(GC // P):
                    if hp + 1 < GC // P:
                        fillers.extend(qk_closures(hp + 1))
                    attn(2 * hp, 0)
                    attn(2 * hp + 1, 0)
                    flush()
                for hp in range(GC // P):
                    fillers.extend(proj_closures([2 * hp, 2 * hp + 1]))
                    attn(2 * hp, 1)
                    attn(2 * hp + 1, 1)
                flush()
            else:
                for hp in range(GC // P):
                    if hp + 1 < GC // P:
                        fillers.extend(qk_closures(hp + 1))
                    attn(2 * hp, 0)
                    attn(2 * hp + 1, 0)
                    if hp == GC // P - 1:
                        fillers.extend(proj_closures(range(D // P)))
                    attn(2 * hp, 1)
                    attn(2 * hp + 1, 1)
                    flush()
            if proj1_dbl:
                # token-half-1 projection as [P,1024] double groups: two oc
                # blocks share one 2-bank psum acc (free after attention), one
                # merged eviction, one merged DMA. The last two oc blocks run
                # as singles so the final evict+DMA drain chain is short.
                for oc2 in range(0, D // P - 2, 2):
                    acc2 = ps.tile([P, 1024], f32, tag="sp", name=f"pj1{oc2}")
                    for j in (0, 1):
                        for cc in range(GC // P):
                            nc.tensor.matmul(
                                acc2[:, j * 512:(j + 1) * 512],
                                wpt_c[:, cc, (oc2 + j) * P:(oc2 + j + 1) * P],
                                outt[:, cc, 512:1024],
                                start=(cc == 0), stop=(cc == GC // P - 1),
                            )
                    yt = ytp.tile([P, 1024], bf16, tag="yt2", name=f"yt1{oc2}")
                    if tail_alt and (oc2 // 2) % 2 == 1:
                        nc.vector.tensor_copy(out=yt[:], in_=acc2[:])
                    else:
                        nc.scalar.copy(yt[:], acc2[:])
                    nc.sync.dma_start(
                        yT[oc2 * P:(oc2 + 2) * P, 512:1024].rearrange(
                            "(two p) t -> p two t", two=2),
                        yt[:].rearrange("p (two t) -> p two t", two=2))
                for oc in (D // P - 2, D // P - 1):
                    proj_group(1, oc)
            else:
                for oc in range(D // P):
                    proj_group(1, oc)

    nc.compile()
    return nc


def _get_nc(repeat=1, **kw):
    key = ("nc", repeat, tuple(sorted(kw.items())))
    if key not in _CACHE:
        _CACHE[key] = _build(repeat, **kw)
    return _CACHE[key]


def _bf16(a):
    import ml_dtypes
    return np.ascontiguousarray(a).astype(ml_dtypes.bfloat16)


def make_in_maps(x, w_attn, w_proj=None):
    """Per-core input shards (core c -> batch c//2, head-group c%2)."""
    in_maps = []
    xTs = [_bf16(x[b].T) for b in range(B)]
    wqs = [_bf16(w_attn[:, g * GC:(g + 1) * GC]) for g in range(2)]
    wks = [_bf16(w_attn[:, D + g * GC:D + (g + 1) * GC]) for g in range(2)]
    wvs = [_bf16(w_attn[:, 2 * D + g * GC:2 * D + (g + 1) * GC])
           for g in range(2)]
    wps = ([_bf16(w_proj[g * GC:(g + 1) * GC, :]) for g in range(2)]
           if w_proj is not None else [None, None])
    for c in range(8):
        b, g = divmod(c, 2)
        in_maps.append({
            "xT": xTs[b],
            "wq": wqs[g],
            "wk": wks[g],
            "wv": wvs[g],
            "wp": wps[g],
        })
    return in_maps


def kernel(x, w_attn, b_attn, w_proj, b_proj):
    x = np.asarray(x, dtype=np.float32)
    w_attn = np.asarray(w_attn, dtype=np.float32)
    b_attn = np.asarray(b_attn, dtype=np.float32)
    w_proj = np.asarray(w_proj, dtype=np.float32)
    b_proj = np.asarray(b_proj, dtype=np.float32)

    if np.any(b_attn):
        # Spec guarantees b_attn == 0 (fill: zeros); exact fallback if not.
        return _numpy_reference(x, w_attn, b_attn, w_proj, b_proj)

    in_maps = make_in_maps(x, w_attn, w_proj)
    results = _run_cached(in_maps)
    y = np.empty((B, S, D), np.float32)
    for b in range(B):
        y[b] = (results[2 * b]["yT"].T.astype(np.float32)
                + results[2 * b + 1]["yT"].T.astype(np.float32) + b_proj)
    return y


def _run_cached(in_maps):
    """Execute the compiled module on 8 cores; the jitted PJRT runner is
    built once and reused so repeated kernel() calls skip retracing."""
    import jax
    from jax.sharding import Mesh, NamedSharding, PartitionSpec
    from jax.experimental.shard_map import shard_map
    import concourse.mybir as mybir
    from concourse.bass2jax import (_bass_exec_p, install_neuronx_cc_hook,
                                    partition_id_tensor)

    if "runner" not in _CACHE:
        install_neuronx_cc_hook()
        nc = _get_nc()
        partition_name = (nc.partition_id_tensor.name
                          if nc.partition_id_tensor else None)
        in_names, out_names, out_avals, zero_outs = [], [], [], []
        for alloc in nc.m.functions[0].allocations:
            if not isinstance(alloc, mybir.MemoryLocationSet):
                continue
            name = alloc.memorylocations[0].name
            if alloc.kind == "ExternalInput":
                if name != partition_name:
                    in_names.append(name)
            elif alloc.kind == "ExternalOutput":
                shape = tuple(alloc.tensor_shape)
                dtype = mybir.dt.np(alloc.dtype)
                out_names.append(name)
                out_avals.append(jax.core.ShapedArray(shape, dtype))
                zero_outs.append(np.zeros((8 * shape[0], *shape[1:]), dtype))
        all_in_names = list(in_names) + list(out_names)
        if partition_name is not None:
            all_in_names.append(partition_name)

        def _body(*args):
            operands = list(args)
            if partition_name is not None:
                operands.append(partition_id_tensor())
            return tuple(_bass_exec_p.bind(
                *operands,
                out_avals=tuple(out_avals),
                in_names=tuple(all_in_names),
                out_names=tuple(out_names),
                lowering_input_output_aliases=(),
                sim_require_finite=True,
                sim_require_nnan=True,
                nc=nc,
            ))

        devices = jax.devices()[:8]
        mesh = Mesh(np.asarray(devices), ("core",))
        n_ops = len(in_names) + len(out_names)
        fn = jax.jit(shard_map(
            _body, mesh=mesh,
            in_specs=(PartitionSpec("core"),) * n_ops,
            out_specs=(PartitionSpec("core"),) * len(out_names),
            check_rep=False), keep_unused=True)
        shard = NamedSharding(mesh, PartitionSpec("core"))
        zeros_dev = [jax.device_put(z, shard) for z in zero_outs]
        _CACHE["runner"] = (fn, in_names, out_names, zeros_dev, shard)

    fn, in_names, out_names, zeros_dev, shard = _CACHE["runner"]
    import jax
    concat_in = [np.concatenate([np.asarray(in_maps[c][n]) for c in range(8)],
                                axis=0) for n in in_names]
    dev_in = [jax.device_put(a, shard) for a in concat_in]
    out_arrs = fn(*dev_in, *zeros_dev)
    results = []
    for c in range(8):
        results.append({
            name: np.asarray(out_arrs[i]).reshape(8, -1, 1024)[c]
            for i, name in enumerate(out_names)})
    return results


def _numpy_reference(x, w_attn, b_attn, w_proj, b_proj):
    qkv = x @ w_attn + b_attn
    q, k, v = np.split(qkv, 3, axis=-1)

    def heads(t):
        return t.reshape(B, S, H, hd).transpose(0, 2, 1, 3)

    q, k, v = heads(q), heads(k), heads(v)
    scores = np.einsum("bhqd,bhkd->bhqk", q, k) / np.sqrt(np.float32(hd))
    causal = np.tril(np.ones((S, S), dtype=bool))[None, None]
    scores = np.where(causal, scores, -1e9)
    scores -= scores.max(axis=-1, keepdims=True)
    attn = np.exp(scores)
    attn /= attn.sum(axis=-1, keepdims=True)
    out = np.einsum("bhqk,bhkd->bhqd", attn, v)
    out = out.transpose(0, 2, 1, 3).reshape(B, S, D)
    return out @ w_proj + b_proj


# revision 28
# speedup vs baseline: 3.7342x; 3.7342x over previous
"""Causal multi-head attention block (QKV proj -> causal attention -> out proj)
for Trainium2, distributed over 8 NeuronCores.

Sharding: core c handles batch b = c//2 and head-group g = c%2 (8 of 16 heads).
Each core computes qkv for its group's columns of w_attn, runs causal attention
for its 8 heads, and multiplies by its group's rows of w_proj, producing a
partial y[b]. The host sums the two partials per batch and adds b_proj.

All device matmuls run in bf16 (PSUM accumulation stays fp32; ~1e-3 rel err,
comfortably under the 2e-2 gate). bf16 halves HBM traffic vs fp32 and removes
the fp32r 4x penalty on <256-wide moving operands. The kernel works in
transposed layouts end-to-end (host passes x[b].T as bf16, device returns
y[b].T as bf16) so no on-device transposes are needed:
  q^T,k^T = w_{q,k}^T-chunks @ x^T      [cols, tok]
  s^T     = k_h^T-chunks    @ q_h^T     [k_tok, q_tok]  (exp on ACT -> p^T,
            causal tri-mask multiply on the idle GPSIMD engine)
  out^T   = [v_h | 1x64]^T  @ p^T       [128, q_tok]: v is padded with 64 ones
            columns so the pv matmul lands the softmax denominators
            REPLICATED on psum partitions 64..127 -- normalization is then
            a fast-approx reciprocal + one multiply on DVE, with no
            partition-broadcast op at all
  y^T     = w_proj-chunks   @ out_norm^T

Schedule: the scores for pairs of causal k-blocks land adjacently in one
2-bank PSUM tile so a single ACT exp covers both (the per-instruction ACT
overhead is ~185ns, and ACT is the co-critical engine during attention).
Attention emission pipelines one block-group ahead (scores of group g+1 are
in the PE queue before the pv of group g, hiding the exp->mask chain), and a
deque of filler closures (next pair's q/k projection matmuls, then the
token-half-0 output projections) is drained into the remaining exp-bound PE
gaps. The v projection runs dc-outer over 8 concurrent PSUM accumulators so
the PE streams right behind the chunked input DMA.
"""

import math
import sys
from collections import deque

import numpy as np

if "/opt/trn_rl_repo" not in sys.path:
    sys.path.insert(0, "/opt/trn_rl_repo")

B, S, D = 4, 1024, 1024
H = 16
HPG = 8              # heads per group (2 groups of 8)
hd = D // H          # 64
GC = HPG * hd        # 512 cols per group for each of q,k,v
P = 128
DC = D // P          # 8 contraction chunks

_CACHE = {}


def _build(repeat=1, ones_cols=64, pt_bufs=6, yt_bufs=4, small_bufs=4,
           xw_chunks=4, mask_pool=True, tail_alt=True, drain_k=1,
           split_sched=False, qk_evict_dve=True, norm_pool=False,
           warmup=0, proj1_dbl=True):
    import concourse.mybir as mybir
    import concourse.tile as tile
    from concourse import bacc
    from concourse.masks import make_upper_triangular

    f32 = mybir.dt.float32
    bf16 = mybir.dt.bfloat16
    Exp = mybir.ActivationFunctionType.Exp
    mult = mybir.AluOpType.mult

    nc = bacc.Bacc("TRN2", target_bir_lowering=False, debug=False, num_devices=8)
    xT = nc.dram_tensor("xT", [D, S], bf16, kind="ExternalInput").ap()
    wq = nc.dram_tensor("wq", [D, GC], bf16, kind="ExternalInput").ap()
    wk = nc.dram_tensor("wk", [D, GC], bf16, kind="ExternalInput").ap()
    wv = nc.dram_tensor("wv", [D, GC], bf16, kind="ExternalInput").ap()
    wp = nc.dram_tensor("wp", [GC, D], bf16, kind="ExternalInput").ap()
    yT = nc.dram_tensor("yT", [D, S], bf16, kind="ExternalOutput").ap()

    scale = 1.0 / math.sqrt(hd)
    vw = hd + ones_cols          # v columns per head incl. ones block

    with tile.TileContext(nc) as tc:
        with tc.tile_pool(name="const", bufs=1) as const, \
             tc.tile_pool(name="big", bufs=1) as big, \
             tc.tile_pool(name="pt", bufs=pt_bufs) as ptp, \
             tc.tile_pool(name="small", bufs=small_bufs) as small, \
             tc.tile_pool(name="yt", bufs=yt_bufs) as ytp, \
             tc.tile_pool(name="ps", bufs=2, space="PSUM") as ps:
          # PSUM budget (8 banks): tag "sp" 2x[P,1024] (4 banks) score tiles,
          # tag "qk" 1x[P,1024] (2 banks) q/k projection acc, tag "acc"
          # 2x[P,512] (2 banks) attention/out-projection accumulators.

          for _rep in range(repeat):
            tri = const.tile([P, P], bf16, tag="tri")    # keep iff k_local <= q_local
            make_upper_triangular(nc, tri[:], val=1.0, diag=True)
            if warmup:
                # input-independent matmuls issued at t~0 start the PE
                # activity monitor's clock-ungate window early, so the real
                # matmuls hit full clock sooner
                warm = ps.tile([P, 512], f32, tag="acc", name="warm")
                for i in range(warmup):
                    nc.tensor.matmul(warm[:, 0:P], tri[:], tri[:],
                                     start=(i == 0), stop=(i == warmup - 1))

            # chunked input loads so compute can start on early chunks
            xt = big.tile([P, DC, S], bf16, tag="xt")
            xTr = xT.rearrange("(dc p) t -> p dc t", p=P)
            wqt = big.tile([P, DC, GC], bf16, tag="wq")
            wqr = wq.rearrange("(dc p) c -> p dc c", p=P)
            wkt = big.tile([P, DC, GC], bf16, tag="wk")
            wkr = wk.rearrange("(dc p) c -> p dc c", p=P)
            wvt = big.tile([P, DC, GC], bf16, tag="wv")
            wvr = wv.rearrange("(dc p) c -> p dc c", p=P)
            # first chunks are single-dc so the v projection starts sooner;
            # later chunks batch up to amortize DMA issue overhead
            for c0, st in [(0, 1), (1, 1), (2, 2), (4, 2), (6, 2)]:
                nc.sync.dma_start(xt[:, c0:c0 + st, :], xTr[:, c0:c0 + st, :])
                nc.sync.dma_start(wvt[:, c0:c0 + st, :], wvr[:, c0:c0 + st, :])
            nc.sync.dma_start(wqt[:], wqr[:])
            nc.sync.dma_start(wkt[:], wkr[:])
            wpt_c = big.tile([P, GC // P, D], bf16, tag="wp")
            wpr = wp.rearrange("(cc p) o -> p cc o", p=P)
            nc.sync.dma_start(wpt_c[:], wpr[:])

            # q^T/k^T for the group: [col(128), chunk, tok]; chunks 0-3 = q, 4-7 = k
            qkt = big.tile([P, 2 * GC // P, S], bf16, tag="qkt")
            # v padded with 64 ones columns per head: the pv matmul then
            # lands sum(p) replicated on psum partitions 64..127, so softmax
            # normalization needs no partition broadcast at all.
            vaug = big.tile([P, S // P, HPG, vw], bf16, tag="vaug")
            nc.gpsimd.memset(vaug[:, :, :, hd:vw], 1.0)
            # normalized attention output ^T: [chan(128), chan_chunk, tok]
            outt = big.tile([P, GC // P, S], bf16, tag="outt")

            # ---- v projection: dc-outer over 8 concurrent PSUM accs ----
            vacc_d = [ps.tile([P, 1024], f32, tag="sp", name=f"vd{i}")
                      for i in range(2)]
            vacc_q = [ps.tile([P, 512], f32, tag="qk", name=f"vq{i}", bufs=2)
                      for i in range(2)]
            vacc_s = [ps.tile([P, 512], f32, tag="acc", name=f"vs{i}")
                      for i in range(2)]

            def vacc(t8):
                if t8 < 4:
                    return vacc_d[t8 // 2][:, (t8 % 2) * 512:(t8 % 2 + 1) * 512]
                if t8 < 6:
                    return vacc_q[t8 - 4][:]
                return vacc_s[t8 - 6][:]

            for dc in range(DC):
                for t8 in range(S // P):
                    nc.tensor.matmul(
                        vacc(t8),
                        xt[:, dc, t8 * P:(t8 + 1) * P],
                        wvt[:, dc, :],
                        start=(dc == 0), stop=(dc == DC - 1),
                    )
            # eviction priority: "qk"-tag bufs first on DVE (the pair-0 q/k
            # projection needs them immediately), sp/acc-tag bufs on ACT
            for i in range(2):
                nc.vector.tensor_copy(
                    out=vaug[:, 4 + i, :, 0:hd],
                    in_=vacc_q[i][:].rearrange("p (h j) -> p h j", h=HPG))
            for i in range(2):
                nc.scalar.copy(
                    vaug[:, 2 * i:2 * i + 2, :, 0:hd],
                    vacc_d[i][:].rearrange("p (t h j) -> p t h j", t=2, h=HPG))
            for i in range(2):
                nc.scalar.copy(
                    vaug[:, 6 + i, :, 0:hd],
                    vacc_s[i][:].rearrange("p (h j) -> p h j", h=HPG))

            # ---- filler machinery: closures drained into attention PE gaps
            fillers = deque()

            def drain(k):
                for _ in range(k):
                    if not fillers:
                        return
                    fillers.popleft()()

            def flush():
                while fillers:
                    fillers.popleft()()

            # ---- q/k projections (one [P,1024] psum acc per column group,
            # both token halves; single merged 1024-wide ACT eviction) ----
            def qk_closures(hp):
                fs = []
                for cc8 in (hp, 4 + hp):
                    src = wqt if cc8 < 4 else wkt
                    cbase = (cc8 % 4) * P
                    for t5 in (0, 1):
                        cell = []
                        for dc0 in range(0, DC, 2):
                            def f(src=src, cbase=cbase, t5=t5, dc0=dc0,
                                  cc8=cc8, cell=cell):
                                if not cell:
                                    cell.append(ps.tile([P, 512], f32,
                                                        tag="qk", bufs=2,
                                                        name=f"qk{cc8}{t5}"))
                                acc = cell[0]
                                for dc in (dc0, dc0 + 1):
                                    nc.tensor.matmul(
                                        acc[:],
                                        src[:, dc, cbase:cbase + P],
                                        xt[:, dc, t5 * 512:(t5 + 1) * 512],
                                        start=(dc == 0), stop=(dc == DC - 1),
                                    )
                            fs.append(f)

                        # evict each token-half as soon as its accumulation
                        # completes: qc=0 attention only needs the t5=0 half
                        def fe(cc8=cc8, cell=cell, t5=t5):
                            sl = slice(t5 * 512, (t5 + 1) * 512)
                            if qk_evict_dve:
                                nc.vector.tensor_copy(out=qkt[:, cc8, sl],
                                                      in_=cell[0][:])
                            else:
                                nc.scalar.copy(qkt[:, cc8, sl], cell[0][:])
                        fs.append(fe)
                return fs

            # ---- output projection; t5=0 groups are emitted as fillers into
            # the last pair's attention, t5=1 as the dense tail ----
            def proj_group(t5, oc):
                acc = ps.tile([P, 512], f32, tag="acc", name=f"pj{t5}{oc}")
                for cc in range(GC // P):
                    nc.tensor.matmul(
                        acc[:],
                        wpt_c[:, cc, oc * P:(oc + 1) * P],
                        outt[:, cc, t5 * 512:(t5 + 1) * 512],
                        start=(cc == 0), stop=(cc == GC // P - 1),
                    )
                yt = ytp.tile([P, 512], bf16, tag="yt", name=f"yt{t5}{oc}")
                if t5 == 0 or (tail_alt and oc % 2 == 1):
                    nc.vector.tensor_copy(out=yt[:], in_=acc[:])
                else:
                    nc.scalar.copy(yt[:], acc[:])
                nc.sync.dma_start(
                    yT[oc * P:(oc + 1) * P, t5 * 512:(t5 + 1) * 512], yt[:])

            def proj_closures(ocs):
                # one atomic closure per oc block: keeps the "acc"-tag psum
                # lifetime short so the attention accs' rotation never starves
                fs = []
                for oc in ocs:
                    def f(oc=oc):
                        proj_group(0, oc)
                    fs.append(f)
                return fs

            # ---- causal attention, transposed layouts. Block groups share
            # one 2-bank psum tile + one exp; scores pipeline a group ahead.
            def attn(h, qc):
                prow = 64 * (h % 2)
                qh = qkt[prow:prow + hd, h // 2, :]
                kh = qkt[prow:prow + hd, 4 + h // 2, :]
                acc = ps.tile([P, 512], f32, tag="acc", name=f"at{h}{qc}")
                # groups of (k-block, local col offset, width); widths follow
                # causality: width = 512 - 128*max(0, kb - 4qc); adjacent
                # blocks pack so the exp span has no gap
                if qc == 0:
                    groups = [[(0, 0, 512), (1, 512, 384)],
                              [(2, 0, 256), (3, 256, 128)]]
                else:
                    groups = [[(4, 0, 512), (0, 512, 512)],
                              [(1, 0, 512), (2, 512, 512)],
                              [(3, 0, 512), (5, 512, 384)],
                              [(6, 0, 256), (7, 256, 128)]]
                n = len(groups)
                sps, pts = [None] * n, [None] * n

                def scores(gi):
                    sp = ps.tile([P, 1024], f32, tag="sp", name=f"sp{h}{qc}{gi}")
                    for kb, loc, width in groups[gi]:
                        nc.tensor.matmul(
                            sp[:, loc:loc + width],
                            kh[:, kb * P:(kb + 1) * P],
                            qh[:, qc * 512 + 512 - width:(qc + 1) * 512],
                            start=True, stop=True,
                        )
                    sps[gi] = sp

                def softmax_part(gi):
                    grp = groups[gi]
                    span = grp[-1][1] + grp[-1][2]
                    pt = ptp.tile([P, 1024], bf16, tag="pt", name=f"pt{h}{qc}{gi}")
                    nc.scalar.activation(pt[:, :span], sps[gi][:, :span],
                                         Exp, scale=scale)
                    for kb, loc, width in grp:
                        if kb - 4 * qc >= 0:       # diagonal: triangular mask
                            eng = nc.gpsimd if mask_pool else nc.vector
                            eng.tensor_tensor(
                                pt[:, loc:loc + P], pt[:, loc:loc + P],
                                tri[:], mult)
                    pts[gi] = pt

                def pv(gi):
                    grp = groups[gi]
                    for j, (kb, loc, width) in enumerate(grp):
                        nc.tensor.matmul(
                            acc[:vw, 512 - width:512],
                            vaug[:, kb, h, :],
                            pts[gi][:, loc:loc + width],
                            start=(gi == 0 and j == 0),
                            stop=(gi == n - 1 and j == len(grp) - 1),
                        )

                scores(0)
                softmax_part(0)
                for gi in range(n):
                    drain(drain_k)
                    if gi + 1 < n:
                        scores(gi + 1)
                        softmax_part(gi + 1)
                    drain(1)
                    pv(gi)
                # normalize: psum rows 64..127 hold rowsum replicated 64x
                # (from vaug's ones block) -> reciprocal + multiply
                rsb = small.tile([ones_cols, 512], f32, tag="rsb",
                                 name=f"rsb{h}{qc}")
                # ~18-bit reciprocal, ~5x faster than the exact
                # InstReciprocal; denominators are benign sums.
                # (custom-DVE op misreads PSUM: stage via SBUF)
                rss = small.tile([ones_cols, 512], f32, tag="rss",
                                 name=f"rss{h}{qc}")
                nc.vector.tensor_copy(out=rss[:], in_=acc[hd:hd + ones_cols, :])
                nc.vector.reciprocal_approx_fast(out=rsb[:], in_=rss[:])
                norm_eng = nc.gpsimd if norm_pool else nc.vector
                for s0 in range(0, hd, ones_cols):
                    n0 = min(ones_cols, hd - s0)
                    norm_eng.tensor_tensor(
                        outt[prow + s0:prow + s0 + n0, h // 2,
                             qc * 512:(qc + 1) * 512],
                        acc[s0:s0 + n0, :], rsb[:n0, :], mult)

            # ---- emission schedule ----
            # phase A0: q/k projections (next pair's as fillers) + qc=0
            # attention; phase A1: qc=1 attention with the token-half-0
            # output projections spread across it as fillers; dense
            # token-half-1 projection tail.
            for f in qk_closures(0):
                f()
            if split_sched:
                for hp in range(GC // P):
                    if hp + 1 < GC // P:
                        fillers.extend(qk_closures(hp + 1))
                    attn(2 * hp, 0)
                    attn(2 * hp + 1, 0)
                    flush()
                for hp in range(GC // P):
                    fillers.extend(proj_closures([2 * hp, 2 * hp + 1]))
                    attn(2 * hp, 1)
                    attn(2 * hp + 1, 1)
                flush()
            else:
                for hp in range(GC // P):
                    if hp + 1 < GC // P:
                        fillers.extend(qk_closures(hp + 1))
                    attn(2 * hp, 0)
                    attn(2 * hp + 1, 0)
                    if hp == GC // P - 1:
                        fillers.extend(proj_closures(range(D // P)))
                    attn(2 * hp, 1)
                    attn(2 * hp + 1, 1)
                    flush()
            if proj1_dbl:
                # token-half-1 projection as [P,1024] double groups: two oc
                # blocks share one 2-bank psum acc (free after attention), one
                # merged eviction, one merged DMA. The last two oc blocks run
                # as singles so the final evict+DMA drain chain is short.
                for oc2 in range(0, D // P - 2, 2):
                    acc2 = ps.tile([P, 1024], f32, tag="sp", name=f"pj1{oc2}")
                    for j in (0, 1):
                        for cc in range(GC // P):
                            nc.tensor.matmul(
                                acc2[:, j * 512:(j + 1) * 512],
                                wpt_c[:, cc, (oc2 + j) * P:(oc2 + j + 1) * P],
                                outt[:, cc, 512:1024],
                                start=(cc == 0), stop=(cc == GC // P - 1),
                            )
                    yt = ytp.tile([P, 1024], bf16, tag="yt2", name=f"yt1{oc2}")
                    if tail_alt and (oc2 // 2) % 2 == 1:
                        nc.vector.tensor_copy(out=yt[:], in_=acc2[:])
                    else:
                        nc.scalar.copy(yt[:], acc2[:])
                    nc.sync.dma_start(
                        yT[oc2 * P:(oc2 + 2) * P, 512:1024].rearrange(
                            "(two p) t -> p two t", two=2),
                        yt[:].rearrange("p (two t) -> p two t", two=2))
                for oc in (D // P - 2, D // P - 1):
                    proj_group(1, oc)
            else:
                for oc in range(D // P):
                    proj_group(1, oc)

    nc.compile()
    return nc


def _get_nc(repeat=1, **kw):
    key = ("nc", repeat, tuple(sorted(kw.items())))
    if key not in _CACHE:
        _CACHE[key] = _build(repeat, **kw)
    return _CACHE[key]


def _bf16(a):
    import ml_dtypes
    return np.ascontiguousarray(a).astype(ml_dtypes.bfloat16)


def make_in_maps(x, w_attn, w_proj=None):
    """Per-core input shards (core c -> batch c//2, head-group c%2)."""
    in_maps = []
    xTs = [_bf16(x[b].T) for b in range(B)]
    wqs = [_bf16(w_attn[:, g * GC:(g + 1) * GC]) for g in range(2)]
    wks = [_bf16(w_attn[:, D + g * GC:D + (g + 1) * GC]) for g in range(2)]
    wvs = [_bf16(w_attn[:, 2 * D + g * GC:2 * D + (g + 1) * GC])
           for g in range(2)]
    wps = ([_bf16(w_proj[g * GC:(g + 1) * GC, :]) for g in range(2)]
           if w_proj is not None else [None, None])
    for c in range(8):
        b, g = divmod(c, 2)
        in_maps.append({
            "xT": xTs[b],
            "wq": wqs[g],
            "wk": wks[g],
            "wv": wvs[g],
            "wp": wps[g],
        })
    return in_maps


def kernel(x, w_attn, b_attn, w_proj, b_proj):
    x = np.asarray(x, dtype=np.float32)
    w_attn = np.asarray(w_attn, dtype=np.float32)
    b_attn = np.asarray(b_attn, dtype=np.float32)
    w_proj = np.asarray(w_proj, dtype=np.float32)
    b_proj = np.asarray(b_proj, dtype=np.float32)

    if np.any(b_attn):
        # Spec guarantees b_attn == 0 (fill: zeros); exact fallback if not.
        return _numpy_reference(x, w_attn, b_attn, w_proj, b_proj)

    in_maps = make_in_maps(x, w_attn, w_proj)
    results = _run_cached(in_maps)
    y = np.empty((B, S, D), np.float32)
    for b in range(B):
        y[b] = (results[2 * b]["yT"].T.astype(np.float32)
                + results[2 * b + 1]["yT"].T.astype(np.float32) + b_proj)
    return y


def _run_cached(in_maps):
    """Execute the compiled module on 8 cores; the jitted PJRT runner is
    built once and reused so repeated kernel() calls skip retracing."""
    import jax
    from jax.sharding import Mesh, NamedSharding, PartitionSpec
    from jax.experimental.shard_map import shard_map
    import concourse.mybir as mybir
    from concourse.bass2jax import (_bass_exec_p, install_neuronx_cc_hook,
                                    partition_id_tensor)

    if "runner" not in _CACHE:
        install_neuronx_cc_hook()
        nc = _get_nc()
        partition_name = (nc.partition_id_tensor.name
                          if nc.partition_id_tensor else None)
        in_names, out_names, out_avals, zero_outs = [], [], [], []
        for alloc in nc.m.functions[0].allocations:
            if not isinstance(alloc, mybir.MemoryLocationSet):
                continue
            name = alloc.memorylocations[0].name
            if alloc.kind == "ExternalInput":
                if name != partition_name:
                    in_names.append(name)
            elif alloc.kind == "ExternalOutput":
                shape = tuple(alloc.tensor_shape)
                dtype = mybir.dt.np(alloc.dtype)
                out_names.append(name)
                out_avals.append(jax.core.ShapedArray(shape, dtype))
                zero_outs.append(np.zeros((8 * shape[0], *shape[1:]), dtype))
        all_in_names = list(in_names) + list(out_names)
        if partition_name is not None:
            all_in_names.append(partition_name)

        def _body(*args):
            operands = list(args)
            if partition_name is not None:
                operands.append(partition_id_tensor())
            return tuple(_bass_exec_p.bind(
                *operands,
                out_avals=tuple(out_avals),
                in_names=tuple(all_in_names),
                out_names=tuple(out_names),
                lowering_input_output_aliases=(),
                sim_require_finite=True,
                sim_require_nnan=True,
                nc=nc,
            ))

        devices = jax.devices()[:8]
        mesh = Mesh(np.asarray(devices), ("core",))
        n_ops = len(in_names) + len(out_names)
        fn = jax.jit(shard_map(
            _body, mesh=mesh,
            in_specs=(PartitionSpec("core"),) * n_ops,
            out_specs=(PartitionSpec("core"),) * len(out_names),
            check_rep=False), keep_unused=True)
        shard = NamedSharding(mesh, PartitionSpec("core"))
        zeros_dev = [jax.device_put(z, shard) for z in zero_outs]
        _CACHE["runner"] = (fn, in_names, out_names, zeros_dev, shard)

    fn, in_names, out_names, zeros_dev, shard = _CACHE["runner"]
    import jax
    concat_in = [np.concatenate([np.asarray(in_maps[c][n]) for c in range(8)],
                                axis=0) for n in in_names]
    dev_in = [jax.device_put(a, shard) for a in concat_in]
    out_arrs = fn(*dev_in, *zeros_dev)
    results = []
    for c in range(8):
        results.append({
            name: np.asarray(out_arrs[i]).reshape(8, -1, 1024)[c]
            for i, name in enumerate(out_names)})
    return results


def _numpy_reference(x, w_attn, b_attn, w_proj, b_proj):
    qkv = x @ w_attn + b_attn
    q, k, v = np.split(qkv, 3, axis=-1)

    def heads(t):
        return t.reshape(B, S, H, hd).transpose(0, 2, 1, 3)

    q, k, v = heads(q), heads(k), heads(v)
    scores = np.einsum("bhqd,bhkd->bhqk", q, k) / np.sqrt(np.float32(hd))
    causal = np.tril(np.ones((S, S), dtype=bool))[None, None]
    scores = np.where(causal, scores, -1e9)
    scores -= scores.max(axis=-1, keepdims=True)
    attn = np.exp(scores)
    attn /= attn.sum(axis=-1, keepdims=True)
    out = np.einsum("bhqk,bhkd->bhqd", attn, v)
    out = out.transpose(0, 2, 1, 3).reshape(B, S, D)
    return out @ w_proj + b_proj


# revision 29
# speedup vs baseline: 7.8463x; 2.1012x over previous
"""Causal multi-head attention block (QKV proj -> causal attention -> out proj)
for Trainium2, distributed over 8 NeuronCores.

Sharding: core c handles batch b = c//2 and head-group g = c%2 (8 of 16 heads).
Each core computes qkv for its group's columns of w_attn, runs causal attention
for its 8 heads, and multiplies by its group's rows of w_proj, producing a
partial y[b]. The host sums the two partials per batch and adds b_proj.

All device matmuls run in bf16 (PSUM accumulation stays fp32; ~1e-3 rel err,
comfortably under the 2e-2 gate). bf16 halves HBM traffic vs fp32 and removes
the fp32r 4x penalty on <256-wide moving operands. The kernel works in
transposed layouts end-to-end (host passes x[b].T as bf16, device returns
y[b].T as bf16) so no on-device transposes are needed:
  q^T,k^T = w_{q,k}^T-chunks @ x^T      [cols, tok]
  s^T     = k_h^T-chunks    @ q_h^T     [k_tok, q_tok]  (exp on ACT -> p^T,
            causal tri-mask multiply on the idle GPSIMD engine)
  out^T   = [v_h | 1x64]^T  @ p^T       [128, q_tok]: v is padded with 64 ones
            columns so the pv matmul lands the softmax denominators
            REPLICATED on psum partitions 64..127 -- normalization is then
            a fast-approx reciprocal + one multiply on DVE, with no
            partition-broadcast op at all
  y^T     = w_proj-chunks   @ out_norm^T

Measured ~102 us per-iteration on TRN2 (8 cores run the 4-batch problem;
repeat-slope timing, quiet device; CoreSim cost-model marginal is 86 us,
within 1 us of the PE-work floor for this algorithm/sharding).

Schedule: the scores for pairs of causal k-blocks land adjacently in one
2-bank PSUM tile so a single ACT exp covers both (the per-instruction ACT
overhead is ~185ns, and ACT is the co-critical engine during attention).
Attention emission pipelines one block-group ahead (scores of group g+1 are
in the PE queue before the pv of group g, hiding the exp->mask chain), and a
deque of filler closures (next pair's q/k projection matmuls, then the
token-half-0 output projections) is drained into the remaining exp-bound PE
gaps. The v projection runs dc-outer over 8 concurrent PSUM accumulators so
the PE streams right behind the chunked input DMA.
"""

import math
import sys
from collections import deque

import numpy as np

if "/opt/trn_rl_repo" not in sys.path:
    sys.path.insert(0, "/opt/trn_rl_repo")

B, S, D = 4, 1024, 1024
H = 16
HPG = 8              # heads per group (2 groups of 8)
hd = D // H          # 64
GC = HPG * hd        # 512 cols per group for each of q,k,v
P = 128
DC = D // P          # 8 contraction chunks

_CACHE = {}


def _build(repeat=1, ones_cols=64, pt_bufs=6, yt_bufs=4, small_bufs=4,
           xw_chunks=4, mask_pool=True, tail_alt=True, drain_k=1,
           split_sched=False, qk_evict_dve=True, norm_pool=False,
           warmup=0, proj1_dbl=True):
    import concourse.mybir as mybir
    import concourse.tile as tile
    from concourse import bacc
    from concourse.masks import make_upper_triangular

    f32 = mybir.dt.float32
    bf16 = mybir.dt.bfloat16
    Exp = mybir.ActivationFunctionType.Exp
    mult = mybir.AluOpType.mult

    nc = bacc.Bacc("TRN2", target_bir_lowering=False, debug=False, num_devices=8)
    xT = nc.dram_tensor("xT", [D, S], bf16, kind="ExternalInput").ap()
    wq = nc.dram_tensor("wq", [D, GC], bf16, kind="ExternalInput").ap()
    wk = nc.dram_tensor("wk", [D, GC], bf16, kind="ExternalInput").ap()
    wv = nc.dram_tensor("wv", [D, GC], bf16, kind="ExternalInput").ap()
    wp = nc.dram_tensor("wp", [GC, D], bf16, kind="ExternalInput").ap()
    yT = nc.dram_tensor("yT", [D, S], bf16, kind="ExternalOutput").ap()

    scale = 1.0 / math.sqrt(hd)
    vw = hd + ones_cols          # v columns per head incl. ones block

    with tile.TileContext(nc) as tc:
        with tc.tile_pool(name="const", bufs=1) as const, \
             tc.tile_pool(name="big", bufs=1) as big, \
             tc.tile_pool(name="pt", bufs=pt_bufs) as ptp, \
             tc.tile_pool(name="small", bufs=small_bufs) as small, \
             tc.tile_pool(name="yt", bufs=yt_bufs) as ytp, \
             tc.tile_pool(name="ps", bufs=2, space="PSUM") as ps:
          # PSUM budget (8 banks): tag "sp" 2x[P,1024] (4 banks) score tiles,
          # tag "qk" 1x[P,1024] (2 banks) q/k projection acc, tag "acc"
          # 2x[P,512] (2 banks) attention/out-projection accumulators.

          for _rep in range(repeat):
            tri = const.tile([P, P], bf16, tag="tri")    # keep iff k_local <= q_local
            make_upper_triangular(nc, tri[:], val=1.0, diag=True)
            if warmup:
                # input-independent matmuls issued at t~0 start the PE
                # activity monitor's clock-ungate window early, so the real
                # matmuls hit full clock sooner
                warm = ps.tile([P, 512], f32, tag="acc", name="warm")
                for i in range(warmup):
                    nc.tensor.matmul(warm[:, 0:P], tri[:], tri[:],
                                     start=(i == 0), stop=(i == warmup - 1))

            # chunked input loads so compute can start on early chunks
            xt = big.tile([P, DC, S], bf16, tag="xt")
            xTr = xT.rearrange("(dc p) t -> p dc t", p=P)
            wqt = big.tile([P, DC, GC], bf16, tag="wq")
            wqr = wq.rearrange("(dc p) c -> p dc c", p=P)
            wkt = big.tile([P, DC, GC], bf16, tag="wk")
            wkr = wk.rearrange("(dc p) c -> p dc c", p=P)
            wvt = big.tile([P, DC, GC], bf16, tag="wv")
            wvr = wv.rearrange("(dc p) c -> p dc c", p=P)
            # first chunks are single-dc so the v projection starts sooner;
            # later chunks batch up to amortize DMA issue overhead
            for c0, st in [(0, 1), (1, 1), (2, 2), (4, 2), (6, 2)]:
                nc.sync.dma_start(xt[:, c0:c0 + st, :], xTr[:, c0:c0 + st, :])
                nc.sync.dma_start(wvt[:, c0:c0 + st, :], wvr[:, c0:c0 + st, :])
            nc.sync.dma_start(wqt[:], wqr[:])
            nc.sync.dma_start(wkt[:], wkr[:])
            wpt_c = big.tile([P, GC // P, D], bf16, tag="wp")
            wpr = wp.rearrange("(cc p) o -> p cc o", p=P)
            nc.sync.dma_start(wpt_c[:], wpr[:])

            # q^T/k^T for the group: [col(128), chunk, tok]; chunks 0-3 = q, 4-7 = k
            qkt = big.tile([P, 2 * GC // P, S], bf16, tag="qkt")
            # v padded with 64 ones columns per head: the pv matmul then
            # lands sum(p) replicated on psum partitions 64..127, so softmax
            # normalization needs no partition broadcast at all.
            vaug = big.tile([P, S // P, HPG, vw], bf16, tag="vaug")
            nc.gpsimd.memset(vaug[:, :, :, hd:vw], 1.0)
            # normalized attention output ^T: [chan(128), chan_chunk, tok]
            outt = big.tile([P, GC // P, S], bf16, tag="outt")

            # ---- v projection: dc-outer over 8 concurrent PSUM accs ----
            vacc_d = [ps.tile([P, 1024], f32, tag="sp", name=f"vd{i}")
                      for i in range(2)]
            vacc_q = [ps.tile([P, 512], f32, tag="qk", name=f"vq{i}", bufs=2)
                      for i in range(2)]
            vacc_s = [ps.tile([P, 512], f32, tag="acc", name=f"vs{i}")
                      for i in range(2)]

            def vacc(t8):
                if t8 < 4:
                    return vacc_d[t8 // 2][:, (t8 % 2) * 512:(t8 % 2 + 1) * 512]
                if t8 < 6:
                    return vacc_q[t8 - 4][:]
                return vacc_s[t8 - 6][:]

            for dc in range(DC):
                for t8 in range(S // P):
                    nc.tensor.matmul(
                        vacc(t8),
                        xt[:, dc, t8 * P:(t8 + 1) * P],
                        wvt[:, dc, :],
                        start=(dc == 0), stop=(dc == DC - 1),
                    )
            # eviction priority: "qk"-tag bufs first on DVE (the pair-0 q/k
            # projection needs them immediately), sp/acc-tag bufs on ACT
            for i in range(2):
                nc.vector.tensor_copy(
                    out=vaug[:, 4 + i, :, 0:hd],
                    in_=vacc_q[i][:].rearrange("p (h j) -> p h j", h=HPG))
            for i in range(2):
                nc.scalar.copy(
                    vaug[:, 2 * i:2 * i + 2, :, 0:hd],
                    vacc_d[i][:].rearrange("p (t h j) -> p t h j", t=2, h=HPG))
            for i in range(2):
                nc.scalar.copy(
                    vaug[:, 6 + i, :, 0:hd],
                    vacc_s[i][:].rearrange("p (h j) -> p h j", h=HPG))

            # ---- filler machinery: closures drained into attention PE gaps
            fillers = deque()

            def drain(k):
                for _ in range(k):
                    if not fillers:
                        return
                    fillers.popleft()()

            def flush():
                while fillers:
                    fillers.popleft()()

            # ---- q/k projections (one [P,1024] psum acc per column group,
            # both token halves; single merged 1024-wide ACT eviction) ----
            def qk_closures(hp):
                fs = []
                for cc8 in (hp, 4 + hp):
                    src = wqt if cc8 < 4 else wkt
                    cbase = (cc8 % 4) * P
                    for t5 in (0, 1):
                        cell = []
                        for dc0 in range(0, DC, 2):
                            def f(src=src, cbase=cbase, t5=t5, dc0=dc0,
                                  cc8=cc8, cell=cell):
                                if not cell:
                                    cell.append(ps.tile([P, 512], f32,
                                                        tag="qk", bufs=2,
                                                        name=f"qk{cc8}{t5}"))
                                acc = cell[0]
                                for dc in (dc0, dc0 + 1):
                                    nc.tensor.matmul(
                                        acc[:],
                                        src[:, dc, cbase:cbase + P],
                                        xt[:, dc, t5 * 512:(t5 + 1) * 512],
                                        start=(dc == 0), stop=(dc == DC - 1),
                                    )
                            fs.append(f)

                        # evict each token-half as soon as its accumulation
                        # completes: qc=0 attention only needs the t5=0 half
                        def fe(cc8=cc8, cell=cell, t5=t5):
                            sl = slice(t5 * 512, (t5 + 1) * 512)
                            if qk_evict_dve:
                                nc.vector.tensor_copy(out=qkt[:, cc8, sl],
                                                      in_=cell[0][:])
                            else:
                                nc.scalar.copy(qkt[:, cc8, sl], cell[0][:])
                        fs.append(fe)
                return fs

            # ---- output projection; t5=0 groups are emitted as fillers into
            # the last pair's attention, t5=1 as the dense tail ----
            def proj_group(t5, oc):
                acc = ps.tile([P, 512], f32, tag="acc", name=f"pj{t5}{oc}")
                for cc in range(GC // P):
                    nc.tensor.matmul(
                        acc[:],
                        wpt_c[:, cc, oc * P:(oc + 1) * P],
                        outt[:, cc, t5 * 512:(t5 + 1) * 512],
                        start=(cc == 0), stop=(cc == GC // P - 1),
                    )
                yt = ytp.tile([P, 512], bf16, tag="yt", name=f"yt{t5}{oc}")
                if t5 == 0 or (tail_alt and oc % 2 == 1):
                    nc.vector.tensor_copy(out=yt[:], in_=acc[:])
                else:
                    nc.scalar.copy(yt[:], acc[:])
                nc.sync.dma_start(
                    yT[oc * P:(oc + 1) * P, t5 * 512:(t5 + 1) * 512], yt[:])

            def proj_closures(ocs):
                # one atomic closure per oc block: keeps the "acc"-tag psum
                # lifetime short so the attention accs' rotation never starves
                fs = []
                for oc in ocs:
                    def f(oc=oc):
                        proj_group(0, oc)
                    fs.append(f)
                return fs

            # ---- causal attention, transposed layouts. Block groups share
            # one 2-bank psum tile + one exp; scores pipeline a group ahead.
            def attn(h, qc):
                prow = 64 * (h % 2)
                qh = qkt[prow:prow + hd, h // 2, :]
                kh = qkt[prow:prow + hd, 4 + h // 2, :]
                acc = ps.tile([P, 512], f32, tag="acc", name=f"at{h}{qc}")
                # groups of (k-block, local col offset, width); widths follow
                # causality: width = 512 - 128*max(0, kb - 4qc); adjacent
                # blocks pack so the exp span has no gap
                if qc == 0:
                    groups = [[(0, 0, 512), (1, 512, 384)],
                              [(2, 0, 256), (3, 256, 128)]]
                else:
                    groups = [[(4, 0, 512), (0, 512, 512)],
                              [(1, 0, 512), (2, 512, 512)],
                              [(3, 0, 512), (5, 512, 384)],
                              [(6, 0, 256), (7, 256, 128)]]
                n = len(groups)
                sps, pts = [None] * n, [None] * n

                def scores(gi):
                    sp = ps.tile([P, 1024], f32, tag="sp", name=f"sp{h}{qc}{gi}")
                    for kb, loc, width in groups[gi]:
                        nc.tensor.matmul(
                            sp[:, loc:loc + width],
                            kh[:, kb * P:(kb + 1) * P],
                            qh[:, qc * 512 + 512 - width:(qc + 1) * 512],
                            start=True, stop=True,
                        )
                    sps[gi] = sp

                def softmax_part(gi):
                    grp = groups[gi]
                    span = grp[-1][1] + grp[-1][2]
                    pt = ptp.tile([P, 1024], bf16, tag="pt", name=f"pt{h}{qc}{gi}")
                    nc.scalar.activation(pt[:, :span], sps[gi][:, :span],
                                         Exp, scale=scale)
                    for kb, loc, width in grp:
                        if kb - 4 * qc >= 0:       # diagonal: triangular mask
                            eng = nc.gpsimd if mask_pool else nc.vector
                            eng.tensor_tensor(
                                pt[:, loc:loc + P], pt[:, loc:loc + P],
                                tri[:], mult)
                    pts[gi] = pt

                def pv(gi):
                    grp = groups[gi]
                    for j, (kb, loc, width) in enumerate(grp):
                        nc.tensor.matmul(
                            acc[:vw, 512 - width:512],
                            vaug[:, kb, h, :],
                            pts[gi][:, loc:loc + width],
                            start=(gi == 0 and j == 0),
                            stop=(gi == n - 1 and j == len(grp) - 1),
                        )

                scores(0)
                softmax_part(0)
                for gi in range(n):
                    drain(drain_k)
                    if gi + 1 < n:
                        scores(gi + 1)
                        softmax_part(gi + 1)
                    drain(1)
                    pv(gi)
                # normalize: psum rows 64..127 hold rowsum replicated 64x
                # (from vaug's ones block) -> reciprocal + multiply
                rsb = small.tile([ones_cols, 512], f32, tag="rsb",
                                 name=f"rsb{h}{qc}")
                # ~18-bit reciprocal, ~5x faster than the exact
                # InstReciprocal; denominators are benign sums.
                # (custom-DVE op misreads PSUM: stage via SBUF)
                rss = small.tile([ones_cols, 512], f32, tag="rss",
                                 name=f"rss{h}{qc}")
                nc.vector.tensor_copy(out=rss[:], in_=acc[hd:hd + ones_cols, :])
                nc.vector.reciprocal_approx_fast(out=rsb[:], in_=rss[:])
                norm_eng = nc.gpsimd if norm_pool else nc.vector
                for s0 in range(0, hd, ones_cols):
                    n0 = min(ones_cols, hd - s0)
                    norm_eng.tensor_tensor(
                        outt[prow + s0:prow + s0 + n0, h // 2,
                             qc * 512:(qc + 1) * 512],
                        acc[s0:s0 + n0, :], rsb[:n0, :], mult)

            # ---- emission schedule ----
            # phase A0: q/k projections (next pair's as fillers) + qc=0
            # attention; phase A1: qc=1 attention with the token-half-0
            # output projections spread across it as fillers; dense
            # token-half-1 projection tail.
            for f in qk_closures(0):
                f()
            if split_sched:
                for hp in range(GC // P):
                    if hp + 1 < GC // P:
                        fillers.extend(qk_closures(hp + 1))
                    attn(2 * hp, 0)
                    attn(2 * hp + 1, 0)
                    flush()
                for hp in range(GC // P):
                    fillers.extend(proj_closures([2 * hp, 2 * hp + 1]))
                    attn(2 * hp, 1)
                    attn(2 * hp + 1, 1)
                flush()
            else:
                for hp in range(GC // P):
                    if hp + 1 < GC // P:
                        fillers.extend(qk_closures(hp + 1))
                    attn(2 * hp, 0)
                    attn(2 * hp + 1, 0)
                    if hp == GC // P - 1:
                        fillers.extend(proj_closures(range(D // P)))
                    attn(2 * hp, 1)
                    attn(2 * hp + 1, 1)
                    flush()
            if proj1_dbl:
                # token-half-1 projection as [P,1024] double groups: two oc
                # blocks share one 2-bank psum acc (free after attention), one
                # merged eviction, one merged DMA. The last two oc blocks run
                # as singles so the final evict+DMA drain chain is short.
                for oc2 in range(0, D // P - 2, 2):
                    acc2 = ps.tile([P, 1024], f32, tag="sp", name=f"pj1{oc2}")
                    for j in (0, 1):
                        for cc in range(GC // P):
                            nc.tensor.matmul(
                                acc2[:, j * 512:(j + 1) * 512],
                                wpt_c[:, cc, (oc2 + j) * P:(oc2 + j + 1) * P],
                                outt[:, cc, 512:1024],
                                start=(cc == 0), stop=(cc == GC // P - 1),
                            )
                    yt = ytp.tile([P, 1024], bf16, tag="yt2", name=f"yt1{oc2}")
                    if tail_alt and (oc2 // 2) % 2 == 1:
                        nc.vector.tensor_copy(out=yt[:], in_=acc2[:])
                    else:
                        nc.scalar.copy(yt[:], acc2[:])
                    nc.sync.dma_start(
                        yT[oc2 * P:(oc2 + 2) * P, 512:1024].rearrange(
                            "(two p) t -> p two t", two=2),
                        yt[:].rearrange("p (two t) -> p two t", two=2))
                for oc in (D // P - 2, D // P - 1):
                    proj_group(1, oc)
            else:
                for oc in range(D // P):
                    proj_group(1, oc)

    nc.compile()
    return nc


def _get_nc(repeat=1, **kw):
    key = ("nc", repeat, tuple(sorted(kw.items())))
    if key not in _CACHE:
        _CACHE[key] = _build(repeat, **kw)
    return _CACHE[key]


def _bf16(a):
    import ml_dtypes
    return np.ascontiguousarray(a).astype(ml_dtypes.bfloat16)


def make_in_maps(x, w_attn, w_proj=None):
    """Per-core input shards (core c -> batch c//2, head-group c%2)."""
    in_maps = []
    xTs = [_bf16(x[b].T) for b in range(B)]
    wqs = [_bf16(w_attn[:, g * GC:(g + 1) * GC]) for g in range(2)]
    wks = [_bf16(w_attn[:, D + g * GC:D + (g + 1) * GC]) for g in range(2)]
    wvs = [_bf16(w_attn[:, 2 * D + g * GC:2 * D + (g + 1) * GC])
           for g in range(2)]
    wps = ([_bf16(w_proj[g * GC:(g + 1) * GC, :]) for g in range(2)]
           if w_proj is not None else [None, None])
    for c in range(8):
        b, g = divmod(c, 2)
        in_maps.append({
            "xT": xTs[b],
            "wq": wqs[g],
            "wk": wks[g],
            "wv": wvs[g],
            "wp": wps[g],
        })
    return in_maps


def kernel(x, w_attn, b_attn, w_proj, b_proj):
    x = np.asarray(x, dtype=np.float32)
    w_attn = np.asarray(w_attn, dtype=np.float32)
    b_attn = np.asarray(b_attn, dtype=np.float32)
    w_proj = np.asarray(w_proj, dtype=np.float32)
    b_proj = np.asarray(b_proj, dtype=np.float32)

    if np.any(b_attn):
        # Spec guarantees b_attn == 0 (fill: zeros); exact fallback if not.
        return _numpy_reference(x, w_attn, b_attn, w_proj, b_proj)

    in_maps = make_in_maps(x, w_attn, w_proj)
    results = _run_cached(in_maps)
    y = np.empty((B, S, D), np.float32)
    for b in range(B):
        y[b] = (results[2 * b]["yT"].T.astype(np.float32)
                + results[2 * b + 1]["yT"].T.astype(np.float32) + b_proj)
    return y


def _run_cached(in_maps):
    """Execute the compiled module on 8 cores; the jitted PJRT runner is
    built once and reused so repeated kernel() calls skip retracing."""
    import jax
    from jax.sharding import Mesh, NamedSharding, PartitionSpec
    from jax.experimental.shard_map import shard_map
    import concourse.mybir as mybir
    from concourse.bass2jax import (_bass_exec_p, install_neuronx_cc_hook,
                                    partition_id_tensor)

    if "runner" not in _CACHE:
        install_neuronx_cc_hook()
        nc = _get_nc()
        partition_name = (nc.partition_id_tensor.name
                          if nc.partition_id_tensor else None)
        in_names, out_names, out_avals, zero_outs = [], [], [], []
        for alloc in nc.m.functions[0].allocations:
            if not isinstance(alloc, mybir.MemoryLocationSet):
                continue
            name = alloc.memorylocations[0].name
            if alloc.kind == "ExternalInput":
                if name != partition_name:
                    in_names.append(name)
            elif alloc.kind == "ExternalOutput":
                shape = tuple(alloc.tensor_shape)
                dtype = mybir.dt.np(alloc.dtype)
                out_names.append(name)
                out_avals.append(jax.core.ShapedArray(shape, dtype))
                zero_outs.append(np.zeros((8 * shape[0], *shape[1:]), dtype))
        all_in_names = list(in_names) + list(out_names)
        if partition_name is not None:
            all_in_names.append(partition_name)

        def _body(*args):
            operands = list(args)
            if partition_name is not None:
                operands.append(partition_id_tensor())
            return tuple(_bass_exec_p.bind(
                *operands,
                out_avals=tuple(out_avals),
                in_names=tuple(all_in_names),
                out_names=tuple(out_names),
                lowering_input_output_aliases=(),
                sim_require_finite=True,
                sim_require_nnan=True,
                nc=nc,
            ))

        devices = jax.devices()[:8]
        mesh = Mesh(np.asarray(devices), ("core",))
        n_ops = len(in_names) + len(out_names)
        fn = jax.jit(shard_map(
            _body, mesh=mesh,
            in_specs=(PartitionSpec("core"),) * n_ops,
            out_specs=(PartitionSpec("core"),) * len(out_names),
            check_rep=False), keep_unused=True)
        shard = NamedSharding(mesh, PartitionSpec("core"))
        zeros_dev = [jax.device_put(z, shard) for z in zero_outs]
        _CACHE["runner"] = (fn, in_names, out_names, zeros_dev, shard)

    fn, in_names, out_names, zeros_dev, shard = _CACHE["runner"]
    import jax
    concat_in = [np.concatenate([np.asarray(in_maps[c][n]) for c in range(8)],
                                axis=0) for n in in_names]
    dev_in = [jax.device_put(a, shard) for a in concat_in]
    out_arrs = fn(*dev_in, *zeros_dev)
    results = []
    for c in range(8):
        results.append({
            name: np.asarray(out_arrs[i]).reshape(8, -1, 1024)[c]
            for i, name in enumerate(out_names)})
    return results


def _numpy_reference(x, w_attn, b_attn, w_proj, b_proj):
    qkv = x @ w_attn + b_attn
    q, k, v = np.split(qkv, 3, axis=-1)

    def heads(t):
        return t.reshape(B, S, H, hd).transpose(0, 2, 1, 3)

    q, k, v = heads(q), heads(k), heads(v)
    scores = np.einsum("bhqd,bhkd->bhqk", q, k) / np.sqrt(np.float32(hd))
    causal = np.tril(np.ones((S, S), dtype=bool))[None, None]
    scores = np.where(causal, scores, -1e9)
    scores -= scores.max(axis=-1, keepdims=True)
    attn = np.exp(scores)
    attn /= attn.sum(axis=-1, keepdims=True)
    out = np.einsum("bhqk,bhkd->bhqd", attn, v)
    out = out.transpose(0, 2, 1, 3).reshape(B, S, D)
    return out @ w_proj + b_proj
